# revision 1
# baseline (speedup 1.0000x reference)
"""Cross-attention kernel for 8 trn2 NeuronCores.

Problem: B=2, Lq=Lk=2048, D=1024, H=16, dh=64.
  q/k/v = Linear(x); q,k L2-normalized per head; S = q@k.T * 1/8;
  key-pad mask -> -1e9; softmax; mask-aware renorm; eps-smooth toward
  uniform-over-valid; out = attn@v merged -> out_proj.

Sharding: core c handles batch b=c//4, heads [4*(c%4), 4*(c%4)+4)
(two "head pairs" hp of 2 heads each). Each core computes a partial
output-projection over its 256 head dims; the host sums the 8 partials
(4 per batch) and adds the output bias.

Math notes (equivalences used, all within fp rounding of the reference):
  - logits are bounded (|q̂·k̂|/8 <= 0.125) so softmax max-subtraction is
    skipped; masked logits get an additive -30000 bias inside the exp
    (per-key bias = per-partition bias in the transposed S layout), which
    underflows exp to exactly 0 like the reference's -1e9 path.
  - softmax + mask-zero + renorm == (exp @ v) / rowsum(exp) since masked
    entries are exactly 0.
  - eps smoothing: attn' = 0.9*attn + 0.1*valid/nv, so
    out = 0.9*(P@v)/rs + 0.1*vmean, vmean = (valid/nv)@v. The 0.9 is
    folded into the rowsum matmul (lhsT = 1/0.9), vmean*0.1 is computed
    on the host from v_in/Wv/bv exactly.

Device layouts (partition dim first):
  xT     [d_in=128-chunk, tokens]   (host pre-transposes inputs)
  qT/kT  [128 = 2 heads x 64, tokens]  -> S_T matmuls row-packed per head
  v      [tokens, 256]              -> AV matmuls col-packed per head
  S_T    [k-tile=128, q]            -> exp bias = per-partition pad mask
  O_T    [128 = 2 heads x 64, q]    -> feeds out_proj as lhsT directly
"""

import ml_dtypes
import numpy as np

import concourse.bass as bass
from concourse import bacc
import concourse.mybir as mybir
import concourse.tile as tile
from concourse.bass_utils import run_bass_kernel_spmd

F32 = mybir.dt.float32
BF16 = mybir.dt.bfloat16
AF = mybir.ActivationFunctionType

B, L, D = 2, 2048, 1024
H, DH = 16, 64
HEADS_PER_CORE = 4          # -> 256 dims per core, 2 head-pairs
HPC = HEADS_PER_CORE * DH   # 256
SCALE = 0.125               # 1/sqrt(64) / ATTN_TEMP
EPS_SMOOTH = 0.1
INV09 = 1.0 / (1.0 - EPS_SMOOTH)
MASK_BIAS = -30000.0
N_CORES = 8
KT = L // 128               # 16 k tiles
QC = L // 512               # 4 q chunks
NCH = D // 128              # 8 contraction chunks for projections


def _build_nc():
    nc = bacc.Bacc(None)

    xqT = nc.dram_tensor("xqT", [D, L], BF16, kind="ExternalInput")
    xkT = nc.dram_tensor("xkT", [D, L], BF16, kind="ExternalInput")
    xvT = nc.dram_tensor("xvT", [D, L], BF16, kind="ExternalInput")
    wq_t = nc.dram_tensor("wq_t", [D, HPC], BF16, kind="ExternalInput")
    wk_t = nc.dram_tensor("wk_t", [D, HPC], BF16, kind="ExternalInput")
    wv_t = nc.dram_tensor("wv_t", [D, HPC], BF16, kind="ExternalInput")
    wo_t = nc.dram_tensor("wo_t", [HPC, D], BF16, kind="ExternalInput")
    bq = nc.dram_tensor("bq", [2, 1, 128], BF16, kind="ExternalInput")
    bk = nc.dram_tensor("bk", [2, 1, 128], BF16, kind="ExternalInput")
    bv = nc.dram_tensor("bv", [1, HPC], BF16, kind="ExternalInput")
    mbias = nc.dram_tensor("mbias", [128, KT], F32, kind="ExternalInput")
    vmean = nc.dram_tensor("vmean", [2, 128, 1], F32, kind="ExternalInput")
    partial = nc.dram_tensor("partial", [L, D], F32, kind="ExternalOutput")

    with tile.TileContext(nc) as tc:
        with (
            tc.tile_pool(name="consts", bufs=1) as consts,
            tc.tile_pool(name="wpool", bufs=1) as wpool,
            tc.tile_pool(name="persist", bufs=1) as persist,
            tc.tile_pool(name="xstream", bufs=6) as xstream,
            tc.tile_pool(name="xvstream", bufs=8) as xvstream,
            tc.tile_pool(name="l2pool", bufs=4) as l2pool,
            tc.tile_pool(name="ppool", bufs=3) as ppool,
            tc.tile_pool(name="normpool", bufs=4) as normpool,
        ):
            # ---- constants ----
            ones_row = consts.tile([1, 512], BF16, tag="ones_row")
            nc.vector.memset(ones_row, 1.0)
            ones09 = consts.tile([128, 64], BF16, tag="ones09")
            nc.vector.memset(ones09, 1.0)
            blockdiag = consts.tile([128, 128], BF16, tag="blockdiag")
            nc.vector.memset(blockdiag, 0.0)
            nc.vector.memset(blockdiag[0:64, 0:64], 1.0)
            nc.vector.memset(blockdiag[64:128, 64:128], 1.0)
            mbias_sb = consts.tile([128, KT], F32, tag="mbias")
            nc.sync.dma_start(out=mbias_sb, in_=mbias[:, :])
            vmean_sb = []
            for hp in range(2):
                t = consts.tile([128, 1], F32, tag=f"vmean{hp}")
                nc.sync.dma_start(out=t, in_=vmean[hp])
                vmean_sb.append(t)
            bias_sb = {}
            for name, hnd in (("q", bq), ("k", bk)):
                for hp in range(2):
                    t = consts.tile([1, 128], BF16, tag=f"b{name}{hp}")
                    nc.sync.dma_start(out=t, in_=hnd[hp])
                    bias_sb[(name, hp)] = t
            bv_sb = consts.tile([1, HPC], BF16, tag="bv")
            nc.sync.dma_start(out=bv_sb, in_=bv[:, :])

            # ---- weights ----
            # w*_t [D, 256] -> [128, chunk, 256]
            w_sb = {}
            for name, hnd in (("q", wq_t), ("k", wk_t), ("v", wv_t)):
                t = wpool.tile([128, NCH, HPC], BF16, tag=f"w{name}")
                nc.sync.dma_start(
                    out=t, in_=hnd.rearrange("(c p) m -> p c m", p=128)
                )
                w_sb[name] = t
            wo_sb = wpool.tile([128, 2, D], BF16, tag="wo")
            nc.sync.dma_start(
                out=wo_sb, in_=wo_t.rearrange("(h p) m -> p h m", p=128)
            )

            # ---- persistent activations ----
            qTn = [persist.tile([128, L], BF16, tag=f"qTn{hp}", name=f"qTn{hp}")
                   for hp in range(2)]
            kTn = [persist.tile([128, L], BF16, tag=f"kTn{hp}", name=f"kTn{hp}")
                   for hp in range(2)]
            v_sb = persist.tile([128, KT, HPC], BF16, tag="v_sb")
            ofin = [persist.tile([128, L], BF16, tag=f"ofin{hp}", name=f"ofin{hp}")
                    for hp in range(2)]

            # ---- projections ----
            with (
                tc.tile_pool(name="ps_proj", bufs=4, space="PSUM") as ps_proj,
                tc.tile_pool(name="ps_n2", bufs=2, space="PSUM") as ps_n2,
            ):
                # q/k: qT[dout, t] accumulated over d_in chunks
                for name, xhnd, dst in (("q", xqT, qTn), ("k", xkT, kTn)):
                    for qc in range(QC):
                        psums = [
                            ps_proj.tile([128, 512], F32, tag="proj",
                                         name=f"proj{i}")
                            for i in range(2)
                        ]
                        for c in range(NCH):
                            xt = xstream.tile([128, 512], BF16, tag="xt")
                            nc.sync.dma_start(
                                out=xt,
                                in_=xhnd[c * 128:(c + 1) * 128,
                                         qc * 512:(qc + 1) * 512],
                            )
                            for hp in range(2):
                                nc.tensor.matmul(
                                    psums[hp],
                                    lhsT=w_sb[name][:, c, hp * 128:(hp + 1) * 128],
                                    rhs=xt,
                                    start=(c == 0),
                                    stop=False,
                                )
                        for hp in range(2):
                            # + bias (broadcast along tokens via K=1 matmul)
                            nc.tensor.matmul(
                                psums[hp],
                                lhsT=bias_sb[(name, hp)],
                                rhs=ones_row,
                                start=False,
                                stop=True,
                            )
                            # L2 norm over each head's 64 dims
                            sq = l2pool.tile([128, 512], BF16, tag="sq")
                            nc.scalar.square(sq, psums[hp])
                            n2 = ps_n2.tile([128, 512], F32, tag="n2")
                            nc.tensor.matmul(
                                n2, lhsT=blockdiag, rhs=sq, start=True, stop=True
                            )
                            nlen = l2pool.tile([128, 512], F32, tag="nlen")
                            nc.scalar.activation(nlen, n2, AF.Sqrt)
                            rnorm = l2pool.tile([128, 512], F32, tag="rnorm")
                            nc.vector.reciprocal_approx_fast(rnorm, nlen)
                            nc.vector.tensor_mul(
                                dst[hp][:, qc * 512:(qc + 1) * 512],
                                psums[hp], rnorm,
                            )

                # v: v[t, dout] = sum_c xT[c][:, t].T @ w[c]
                for tt in range(KT):
                    vp = ps_proj.tile([128, HPC], F32, tag="proj")
                    for c in range(NCH):
                        xt = xvstream.tile([128, 128], BF16, tag="xvt")
                        nc.sync.dma_start(
                            out=xt,
                            in_=xvT[c * 128:(c + 1) * 128,
                                    tt * 128:(tt + 1) * 128],
                        )
                        nc.tensor.matmul(
                            vp, lhsT=xt, rhs=w_sb["v"][:, c, :],
                            start=(c == 0), stop=False,
                        )
                    nc.tensor.matmul(
                        vp, lhsT=ones_row[:, 0:128], rhs=bv_sb,
                        start=False, stop=True,
                    )
                    nc.vector.tensor_copy(v_sb[:, tt, :], vp)

            # ---- attention ----
            with (
                tc.tile_pool(name="ps_S", bufs=2, space="PSUM") as ps_S,
                tc.tile_pool(name="ps_O", bufs=2, space="PSUM") as ps_O,
                tc.tile_pool(name="ps_rs", bufs=2, space="PSUM") as ps_rs,
            ):
                for hp in range(2):
                    for qc in range(QC):
                        qsl = slice(qc * 512, (qc + 1) * 512)
                        o_ps = ps_O.tile([128, 512], F32, tag="o")
                        rs_ps = ps_rs.tile([128, 512], F32, tag="rs")
                        for kt in range(KT):
                            s_ps = ps_S.tile([128, 1024], F32, tag="s")
                            ksl = slice(kt * 128, (kt + 1) * 128)
                            # S_T = k̂.T q̂ per head, row-packed (K=64 each)
                            nc.tensor.matmul(
                                s_ps[:, 0:512],
                                lhsT=kTn[hp][0:64, ksl],
                                rhs=qTn[hp][0:64, qsl],
                                start=True, stop=True,
                            )
                            nc.tensor.matmul(
                                s_ps[:, 512:1024],
                                lhsT=kTn[hp][64:128, ksl],
                                rhs=qTn[hp][64:128, qsl],
                                start=True, stop=True,
                            )
                            # P = exp(SCALE*S + pad_bias); masked keys -> 0
                            p_sb = ppool.tile([128, 1024], BF16, tag="p")
                            nc.scalar.activation(
                                p_sb, s_ps, AF.Exp,
                                bias=mbias_sb[:, kt:kt + 1], scale=SCALE,
                            )
                            # O_T += v.T @ P, col-packed per head
                            nc.tensor.matmul(
                                o_ps[0:64, :],
                                lhsT=v_sb[:, kt, hp * 128:hp * 128 + 64],
                                rhs=p_sb[:, 0:512],
                                start=(kt == 0), stop=(kt == KT - 1),
                            )
                            nc.tensor.matmul(
                                o_ps[64:128, :],
                                lhsT=v_sb[:, kt, hp * 128 + 64:hp * 128 + 128],
                                rhs=p_sb[:, 512:1024],
                                start=(kt == 0), stop=(kt == KT - 1),
                            )
                            # rs += (1/0.9)*colsum(P), replicated over 64 parts
                            nc.tensor.matmul(
                                rs_ps[0:64, :],
                                lhsT=ones09,
                                rhs=p_sb[:, 0:512],
                                start=(kt == 0), stop=(kt == KT - 1),
                            )
                            nc.tensor.matmul(
                                rs_ps[64:128, :],
                                lhsT=ones09,
                                rhs=p_sb[:, 512:1024],
                                start=(kt == 0), stop=(kt == KT - 1),
                            )
                        # O_final = 0.9*O_T/rs + 0.1*vmean
                        rsb = normpool.tile([128, 512], F32, tag="rsb")
                        nc.vector.tensor_copy(rsb, rs_ps)
                        rr = normpool.tile([128, 512], F32, tag="rr")
                        nc.vector.reciprocal_approx_fast(rr, rsb)
                        om = normpool.tile([128, 512], F32, tag="om")
                        nc.vector.tensor_mul(om, o_ps, rr)
                        nc.vector.tensor_scalar(
                            ofin[hp][:, qsl], om, 1.0 - EPS_SMOOTH,
                            vmean_sb[hp], mybir.AluOpType.mult,
                            mybir.AluOpType.add,
                        )

            # ---- partial output projection ----
            with tc.tile_pool(name="ps_out", bufs=3, space="PSUM") as ps_out:
                for tt in range(KT):
                    tsl = slice(tt * 128, (tt + 1) * 128)
                    for nh in range(2):
                        nsl = slice(nh * 512, (nh + 1) * 512)
                        op = ps_out.tile([128, 512], F32, tag="oproj")
                        nc.tensor.matmul(
                            op, lhsT=ofin[0][:, tsl], rhs=wo_sb[:, 0, nsl],
                            start=True, stop=False,
                        )
                        nc.tensor.matmul(
                            op, lhsT=ofin[1][:, tsl], rhs=wo_sb[:, 1, nsl],
                            start=False, stop=True,
                        )
                        ost = normpool.tile([128, 512], F32, tag="ost")
                        nc.vector.tensor_copy(ost, op)
                        nc.sync.dma_start(out=partial[tsl, nsl], in_=ost)

    nc.finalize()
    return nc


_NC_CACHE = None


def _get_nc():
    global _NC_CACHE
    if _NC_CACHE is None:
        _NC_CACHE = _build_nc()
    return _NC_CACHE


def kernel(q_in, k_in, v_in, kv_pad_mask, Wq, bq, Wk, bk, Wv, bv, Wo, bo,
           _trace=False):
    f32 = np.float32
    q_in = np.asarray(q_in, f32)
    k_in = np.asarray(k_in, f32)
    v_in = np.asarray(v_in, f32)
    mask = np.asarray(kv_pad_mask, bool)
    Wq, bq, Wk, bk, Wv, bv, Wo, bo = (
        np.asarray(a, f32) for a in (Wq, bq, Wk, bk, Wv, bv, Wo, bo)
    )

    nc = _get_nc()

    # per-batch host prep
    xT = {}
    mb = {}
    for b in range(B):
        bf = ml_dtypes.bfloat16
        xT[("q", b)] = np.ascontiguousarray(q_in[b].T).astype(bf)
        xT[("k", b)] = np.ascontiguousarray(k_in[b].T).astype(bf)
        xT[("v", b)] = np.ascontiguousarray(v_in[b].T).astype(bf)
        mb[b] = np.ascontiguousarray(
            np.where(mask[b], MASK_BIAS, 0.0).astype(f32).reshape(KT, 128).T
        )

    in_maps = []
    for core in range(N_CORES):
        b = core // 4
        h0 = (core % 4) * HEADS_PER_CORE
        rows = slice(h0 * DH, h0 * DH + HPC)
        valid = (~mask[b]).astype(f32)
        nv = max(float(valid.sum()), 1.0)
        vscaled = valid * (EPS_SMOOTH / nv)
        # 0.1 * mean_over_valid(v) for this core's 256 dims
        vm = (vscaled @ v_in[b]) @ Wv[rows].T + EPS_SMOOTH * bv[rows]
        in_maps.append({
            "xqT": xT[("q", b)],
            "xkT": xT[("k", b)],
            "xvT": xT[("v", b)],
            "wq_t": np.ascontiguousarray(Wq[rows].T).astype(ml_dtypes.bfloat16),
            "wk_t": np.ascontiguousarray(Wk[rows].T).astype(ml_dtypes.bfloat16),
            "wv_t": np.ascontiguousarray(Wv[rows].T).astype(ml_dtypes.bfloat16),
            "wo_t": np.ascontiguousarray(Wo[:, rows].T).astype(ml_dtypes.bfloat16),
            "bq": np.ascontiguousarray(bq[rows].reshape(2, 1, 128)).astype(ml_dtypes.bfloat16),
            "bk": np.ascontiguousarray(bk[rows].reshape(2, 1, 128)).astype(ml_dtypes.bfloat16),
            "bv": np.ascontiguousarray(bv[rows].reshape(1, HPC)).astype(ml_dtypes.bfloat16),
            "mbias": mb[b],
            "vmean": np.ascontiguousarray(vm.astype(f32).reshape(2, 128, 1)),
        })

    res = run_bass_kernel_spmd(nc, in_maps, core_ids=list(range(N_CORES)),
                               trace=_trace)
    out = np.zeros((B, L, D), f32)
    for core in range(N_CORES):
        out[core // 4] += res.results[core]["partial"]
    out += bo[None, None, :]
    if _trace:
        kernel._last_result = res
    return out



# revision 10
# speedup vs baseline: 2.0149x; 2.0149x over previous
"""Cross-attention kernel for 8 trn2 NeuronCores.

Problem: B=2, Lq=Lk=2048, D=1024, H=16, dh=64.
  q/k/v = Linear(x); q,k L2-normalized per head; S = q@k.T * 1/8;
  key-pad mask -> -1e9; softmax; mask-aware renorm; eps-smooth toward
  uniform-over-valid; out = attn@v merged -> out_proj.

Sharding: core c handles batch b=c//4, heads [4*(c%4), 4*(c%4)+4)
(two "head pairs" hp of 2 heads each). Each core computes a partial
output-projection over its 256 head dims; the host sums the 8 partials
(4 per batch) and adds the output bias.

Key optimizations over the v1 kernel:
  - Key compaction: masked keys contribute exactly 0 (exp underflows to
    0), so the host gathers only the valid keys (~50%) and pads to a
    multiple of 128.  S / exp / AV / k,v projections all shrink
    proportionally.  The compiled graph is cached per padded-key-tile
    count KT_C.
  - Fused rowsum: the AV matmul's stationary matrix is [v_h | ones]
    column-blocks, so PSUM partitions 64:128 accumulate the softmax
    denominator replicated across 64 partitions -- no separate rowsum
    matmuls and no cross-partition broadcast for the divide.
  - Inputs staged with few large DMAs; out-projection interleaved into
    the attention loop; elementwise work split across DVE/ACT/GpSimd.

Math notes (equivalences used, all within fp rounding of the reference):
  - logits are bounded (|q̂·k̂|/8 <= 0.125) so softmax max-subtraction is
    skipped; masked/pad keys get an additive -30000 bias inside the exp
    (per-key bias = per-partition bias in the transposed S layout), which
    underflows exp to exactly 0 like the reference's -1e9 path.
  - softmax + mask-zero + renorm == (exp @ v) / rowsum(exp) since masked
    entries are exactly 0.
  - eps smoothing: attn' = 0.9*attn + 0.1*valid/nv, so
    out = 0.9*(P@v)/rs + 0.1*vmean; vmean*0.1 is computed on the host
    from v_in/Wv/bv exactly.
"""

import ml_dtypes
import numpy as np

import concourse.bass as bass
from concourse import bacc
import concourse.mybir as mybir
import concourse.tile as tile
from concourse.bass_utils import run_bass_kernel_spmd

F32 = mybir.dt.float32
BF16 = mybir.dt.bfloat16
AF = mybir.ActivationFunctionType
ALU = mybir.AluOpType

B, L, D = 2, 2048, 1024
H, DH = 16, 64
HEADS_PER_CORE = 4          # -> 256 dims per core, 2 head-pairs
HPC = HEADS_PER_CORE * DH   # 256
SCALE = 0.125               # 1/sqrt(64) / ATTN_TEMP
EPS_SMOOTH = 0.1
MASK_BIAS = -30000.0
N_CORES = 8
QC = L // 512               # 4 q chunks
NCH = D // 128              # 8 contraction chunks for projections


def _chunks(total, step=512):
    out, s = [], 0
    while s < total:
        cs = min(step, total - s)
        out.append((s, cs))
        s += cs
    return out


def _build_nc(kt_c):
    LKC = kt_c * 128
    nc = bacc.Bacc(None)

    xqT = nc.dram_tensor("xqT", [D, L], BF16, kind="ExternalInput")
    xkT = nc.dram_tensor("xkT", [D, LKC], BF16, kind="ExternalInput")
    xvT = nc.dram_tensor("xvT", [D, LKC], BF16, kind="ExternalInput")
    wq_t = nc.dram_tensor("wq_t", [D, HPC], BF16, kind="ExternalInput")
    wk_t = nc.dram_tensor("wk_t", [D, HPC], BF16, kind="ExternalInput")
    wv_t = nc.dram_tensor("wv_t", [D, HPC], BF16, kind="ExternalInput")
    wo_t = nc.dram_tensor("wo_t", [HPC, D], BF16, kind="ExternalInput")
    bq = nc.dram_tensor("bq", [2, 1, 128], BF16, kind="ExternalInput")
    bk = nc.dram_tensor("bk", [2, 1, 128], BF16, kind="ExternalInput")
    bv = nc.dram_tensor("bv", [1, HPC], BF16, kind="ExternalInput")
    mbias = nc.dram_tensor("mbias", [128, kt_c], F32, kind="ExternalInput")
    vmean = nc.dram_tensor("vmean", [2, 128, 1], F32, kind="ExternalInput")
    partial = nc.dram_tensor("partial", [L, D], F32, kind="ExternalOutput")

    with tile.TileContext(nc) as tc:
        with (
            tc.tile_pool(name="consts", bufs=1) as consts,
            tc.tile_pool(name="wpool", bufs=1) as wpool,
            tc.tile_pool(name="xpool", bufs=1) as xpool,
            tc.tile_pool(name="persist", bufs=1) as persist,
            tc.tile_pool(name="l2pool", bufs=4) as l2pool,
            tc.tile_pool(name="ppool", bufs=3) as ppool,
            tc.tile_pool(name="divpool", bufs=4) as divpool,
            tc.tile_pool(name="ostpool", bufs=3) as ostpool,
            tc.tile_pool(name="ps", bufs=2, space="PSUM") as ps,
        ):
            # ---- weights / consts (DMA issue order matters: k-path first) ----
            w_sb = {}
            for name, hnd in (("k", wk_t), ("v", wv_t), ("q", wq_t)):
                t = wpool.tile([128, NCH, HPC], BF16, tag=f"w{name}",
                               name=f"w{name}")
                nc.sync.dma_start(
                    out=t, in_=hnd.rearrange("(c p) m -> p c m", p=128)
                )
                w_sb[name] = t

            bias_sb = {}
            for name, hnd in (("q", bq), ("k", bk)):
                for hp in range(2):
                    t = consts.tile([1, 128], BF16, tag=f"b{name}{hp}",
                                    name=f"b{name}{hp}")
                    nc.sync.dma_start(out=t, in_=hnd[hp])
                    bias_sb[(name, hp)] = t
            bv_sb = consts.tile([1, HPC], BF16, tag="bv")
            nc.sync.dma_start(out=bv_sb, in_=bv[:, :])
            mbias_sb = consts.tile([128, kt_c], F32, tag="mbias")
            nc.sync.dma_start(out=mbias_sb, in_=mbias[:, :])
            vmean_sb = []
            for hp in range(2):
                t = consts.tile([128, 1], F32, tag=f"vmean{hp}",
                                name=f"vmean{hp}")
                nc.sync.dma_start(out=t, in_=vmean[hp])
                vmean_sb.append(t)

            # staged inputs (one DMA per 128-row chunk of x^T)
            xk_sb = xpool.tile([128, NCH, LKC], BF16, tag="xk")
            xv_sb = xpool.tile([128, NCH, LKC], BF16, tag="xv")
            xq_sb = xpool.tile([128, NCH, L], BF16, tag="xq")
            for c in range(NCH):
                nc.sync.dma_start(out=xk_sb[:, c, :],
                                  in_=xkT[c * 128:(c + 1) * 128, :])
            for c in range(NCH):
                nc.sync.dma_start(out=xv_sb[:, c, :],
                                  in_=xvT[c * 128:(c + 1) * 128, :])
            for c in range(NCH):
                nc.sync.dma_start(out=xq_sb[:, c, :],
                                  in_=xqT[c * 128:(c + 1) * 128, :])
            wo_sb = wpool.tile([128, 2, D], BF16, tag="wo")
            nc.sync.dma_start(
                out=wo_sb, in_=wo_t.rearrange("(h p) m -> p h m", p=128)
            )

            ones_row = consts.tile([1, 512], BF16, tag="ones_row")
            nc.vector.memset(ones_row, 1.0)
            blockdiag = consts.tile([128, 128], BF16, tag="blockdiag")
            nc.vector.memset(blockdiag, 0.0)
            nc.vector.memset(blockdiag[0:64, 0:64], 1.0)
            nc.vector.memset(blockdiag[64:128, 64:128], 1.0)

            # ---- persistent activations ----
            qTn = [persist.tile([128, L], BF16, tag=f"qTn{hp}", name=f"qTn{hp}")
                   for hp in range(2)]
            kTn = [persist.tile([128, LKC], BF16, tag=f"kTn{hp}",
                                name=f"kTn{hp}")
                   for hp in range(2)]
            # [keys, kt, head, 0:64 ones | 64:128 v] -> fused rowsum + AV
            # (rowsum lands on PSUM partitions 0:64 where the reciprocal can
            # read it directly; O lands on 64:128 whose PSUM base is
            # independent of the SBUF operand base in the divide)
            v_aug = persist.tile([128, kt_c, HEADS_PER_CORE, 128], BF16,
                                 tag="v_aug")
            nc.vector.memset(v_aug[:, :, :, 0:64], 1.0)
            ofin = [persist.tile([128, L], BF16, tag=f"ofin{hp}",
                                 name=f"ofin{hp}")
                    for hp in range(2)]

            def proj_block(name, x_sb, dst, ts, cs):
                """Project+L2-normalize one token chunk of q or k."""
                tsl = slice(ts, ts + cs)
                psums = [ps.tile([128, 512], F32, tag="o", bufs=4,
                                 name=f"pj{name}{hp}")
                         for hp in range(2)]
                for c in range(NCH):
                    for hp in range(2):
                        nc.tensor.matmul(
                            psums[hp][:, 0:cs],
                            lhsT=w_sb[name][:, c, hp * 128:(hp + 1) * 128],
                            rhs=x_sb[:, c, tsl],
                            start=(c == 0),
                            stop=False,
                        )
                for hp in range(2):
                    # + bias (broadcast along tokens via K=1 matmul)
                    nc.tensor.matmul(
                        psums[hp][:, 0:cs],
                        lhsT=bias_sb[(name, hp)],
                        rhs=ones_row[:, 0:cs],
                        start=False,
                        stop=True,
                    )
                    # L2 norm over each head's 64 dims
                    sq = l2pool.tile([128, 512], BF16, tag="sq")
                    nc.scalar.square(sq[:, 0:cs], psums[hp][:, 0:cs])
                    n2 = ps.tile([128, 512], F32, tag="s", bufs=2, name="n2")
                    nc.tensor.matmul(
                        n2[:, 0:cs], lhsT=blockdiag, rhs=sq[:, 0:cs],
                        start=True, stop=True,
                    )
                    nlen = l2pool.tile([128, 512], F32, tag="nlen")
                    nc.scalar.sqrt(nlen[:, 0:cs], n2[:, 0:cs])
                    rnorm = l2pool.tile([128, 512], F32, tag="rnorm")
                    nc.vector.reciprocal_approx_fast(rnorm[:, 0:cs],
                                                     nlen[:, 0:cs])
                    nc.vector.tensor_mul(
                        dst[hp][:, tsl], psums[hp][:, 0:cs], rnorm[:, 0:cs],
                    )

            # ---- k projection ----
            for ts, cs in _chunks(LKC):
                proj_block("k", xk_sb, kTn, ts, cs)

            # ---- v projection (into v_aug with ones columns) ----
            for tt in range(kt_c):
                vp = ps.tile([128, HPC], F32, tag="o", bufs=4, name="vp")
                for c in range(NCH):
                    nc.tensor.matmul(
                        vp,
                        lhsT=xv_sb[:, c, tt * 128:(tt + 1) * 128],
                        rhs=w_sb["v"][:, c, :],
                        start=(c == 0), stop=False,
                    )
                nc.tensor.matmul(
                    vp, lhsT=ones_row[:, 0:128], rhs=bv_sb,
                    start=False, stop=True,
                )
                nc.vector.tensor_copy(
                    v_aug[:, tt, :, 64:128],
                    vp.rearrange("p (h d) -> p h d", h=HEADS_PER_CORE),
                )

            # ---- q projection (first chunk up front, rest interleaved) ----
            proj_block("q", xq_sb, qTn, 0, 512)

            # ---- attention + interleaved out-projection ----
            def out_proj(qc):
                for tt4 in range(4):
                    tsl = slice(qc * 512 + tt4 * 128, qc * 512 + tt4 * 128 + 128)
                    for nh in range(2):
                        nsl = slice(nh * 512, (nh + 1) * 512)
                        op = ps.tile([128, 1024], F32, tag="s", bufs=2,
                                     name="op")
                        nc.tensor.matmul(
                            op[:, 0:512], lhsT=ofin[0][:, tsl],
                            rhs=wo_sb[:, 0, nsl], start=True, stop=False,
                        )
                        nc.tensor.matmul(
                            op[:, 0:512], lhsT=ofin[1][:, tsl],
                            rhs=wo_sb[:, 1, nsl], start=False, stop=True,
                        )
                        ost = ostpool.tile([128, 512], F32, tag="ost")
                        nc.vector.tensor_copy(ost, op[:, 0:512])
                        nc.sync.dma_start(out=partial[tsl, nsl], in_=ost)

            for qc in range(QC):
                qsl = slice(qc * 512, (qc + 1) * 512)
                for hp in range(2):
                    o_ps = [ps.tile([128, 512], F32, tag="o", bufs=4,
                                    name=f"o{i}")
                            for i in range(2)]
                    for kt in range(kt_c):
                        ksl = slice(kt * 128, (kt + 1) * 128)
                        s_ps = ps.tile([128, 1024], F32, tag="s", bufs=2,
                                       name="s")
                        # S_T = k̂.T q̂ per head, row-packed (K=64 each)
                        nc.tensor.matmul(
                            s_ps[:, 0:512],
                            lhsT=kTn[hp][0:64, ksl],
                            rhs=qTn[hp][0:64, qsl],
                            start=True, stop=True,
                        )
                        nc.tensor.matmul(
                            s_ps[:, 512:1024],
                            lhsT=kTn[hp][64:128, ksl],
                            rhs=qTn[hp][64:128, qsl],
                            start=True, stop=True,
                        )
                        # P = exp(SCALE*S + pad_bias); masked keys -> 0
                        p_sb = ppool.tile([128, 1024], BF16, tag="p")
                        nc.scalar.activation(
                            p_sb, s_ps, AF.Exp,
                            bias=mbias_sb[:, kt:kt + 1], scale=SCALE,
                        )
                        # O_T (parts 0:64) + replicated rowsum (parts 64:128)
                        for i in range(2):
                            nc.tensor.matmul(
                                o_ps[i],
                                lhsT=v_aug[:, kt, 2 * hp + i, :],
                                rhs=p_sb[:, i * 512:(i + 1) * 512],
                                start=(kt == 0), stop=(kt == kt_c - 1),
                            )
                    # O_final = 0.9*O_T/rs + 0.1*vmean
                    for i in range(2):
                        rr = divpool.tile([128, 512], F32, tag="rr")
                        nc.vector.reciprocal_approx_fast(
                            rr[0:64, :], o_ps[i][0:64, :])
                        t1 = divpool.tile([128, 512], F32, tag="t1")
                        nc.vector.tensor_mul(
                            t1[0:64, :], o_ps[i][64:128, :], rr[0:64, :])
                        nc.gpsimd.tensor_scalar(
                            ofin[hp][64 * i:64 * (i + 1), qsl], t1[0:64, :],
                            1.0 - EPS_SMOOTH,
                            vmean_sb[hp][64 * i:64 * (i + 1), :],
                            ALU.mult, ALU.add,
                        )
                # interleave: q projection for the next chunk, then the
                # out-projection of this chunk (division results drain while
                # the next chunk's attention matmuls keep the PE busy)
                if qc + 1 < QC:
                    proj_block("q", xq_sb, qTn, (qc + 1) * 512, 512)
                out_proj(qc)

    nc.finalize()
    return nc


_NC_CACHE = {}


def _get_nc(kt_c):
    if kt_c not in _NC_CACHE:
        _NC_CACHE[kt_c] = _build_nc(kt_c)
    return _NC_CACHE[kt_c]


def kernel(q_in, k_in, v_in, kv_pad_mask, Wq, bq, Wk, bk, Wv, bv, Wo, bo,
           _trace=False):
    f32 = np.float32
    bf = ml_dtypes.bfloat16
    q_in = np.asarray(q_in, f32)
    k_in = np.asarray(k_in, f32)
    v_in = np.asarray(v_in, f32)
    mask = np.asarray(kv_pad_mask, bool)
    Wq, bq, Wk, bk, Wv, bv, Wo, bo = (
        np.asarray(a, f32) for a in (Wq, bq, Wk, bk, Wv, bv, Wo, bo)
    )

    # key compaction: gather valid keys per batch, pad to a tile multiple
    idx = [np.flatnonzero(~mask[b]) for b in range(B)]
    nv = [len(ix) for ix in idx]
    kt_c = max(1, max((n + 127) // 128 for n in nv))
    LKC = kt_c * 128
    nc = _get_nc(kt_c)

    xT = {}
    mb = {}
    for b in range(B):
        kc = np.zeros((LKC, D), f32)
        kc[:nv[b]] = k_in[b][idx[b]]
        vc = np.zeros((LKC, D), f32)
        vc[:nv[b]] = v_in[b][idx[b]]
        xT[("q", b)] = np.ascontiguousarray(q_in[b].T).astype(bf)
        xT[("k", b)] = np.ascontiguousarray(kc.T).astype(bf)
        xT[("v", b)] = np.ascontiguousarray(vc.T).astype(bf)
        mvalid = np.zeros(LKC, f32)
        mvalid[nv[b]:] = MASK_BIAS
        mb[b] = np.ascontiguousarray(mvalid.reshape(kt_c, 128).T)

    in_maps = []
    for core in range(N_CORES):
        b = core // 4
        h0 = (core % 4) * HEADS_PER_CORE
        rows = slice(h0 * DH, h0 * DH + HPC)
        valid = (~mask[b]).astype(f32)
        nvb = max(float(valid.sum()), 1.0)
        vscaled = valid * (EPS_SMOOTH / nvb)
        # 0.1 * mean_over_valid(v) for this core's 256 dims
        vm = (vscaled @ v_in[b]) @ Wv[rows].T + EPS_SMOOTH * bv[rows]
        in_maps.append({
            "xqT": xT[("q", b)],
            "xkT": xT[("k", b)],
            "xvT": xT[("v", b)],
            "wq_t": np.ascontiguousarray(Wq[rows].T).astype(bf),
            "wk_t": np.ascontiguousarray(Wk[rows].T).astype(bf),
            "wv_t": np.ascontiguousarray(Wv[rows].T).astype(bf),
            "wo_t": np.ascontiguousarray(Wo[:, rows].T).astype(bf),
            "bq": np.ascontiguousarray(bq[rows].reshape(2, 1, 128)).astype(bf),
            "bk": np.ascontiguousarray(bk[rows].reshape(2, 1, 128)).astype(bf),
            "bv": np.ascontiguousarray(bv[rows].reshape(1, HPC)).astype(bf),
            "mbias": mb[b],
            "vmean": np.ascontiguousarray(vm.astype(f32).reshape(2, 128, 1)),
        })

    res = run_bass_kernel_spmd(nc, in_maps, core_ids=list(range(N_CORES)),
                               trace=_trace)
    out = np.zeros((B, L, D), f32)
    for core in range(N_CORES):
        out[core // 4] += res.results[core]["partial"]
    out += bo[None, None, :]
    if _trace:
        kernel._last_result = res
    return out
